# revision 17
# baseline (speedup 1.0000x reference)
"""Trainium2 Bass kernel for nn_ADAPT_19748259627479 (PaCo-style loss_fn).

Strategy (8 NeuronCores, data-parallel over N=V*B=4096):
  - Each core owns a 512-row shard of feats (view-major) and computes the
    heavy [512,5000] work on-device: feats@protos.T matmuls, exp(./eps),
    softplus logits, the total_logits/imp elementwise chain + row sum-exp,
    and a 625-row shard of the proto-contrast exp(pn@pn.T/tau) reductions.
  - One NEFF (beta passed as an input column, so nothing input-dependent is
    baked in), compiled once per process and executed twice: pass 1 with
    protos (yields E1 for Sinkhorn 1), pass 2 with protos2 (yields E2, row
    sumexp for mle's neg term, and the proto-contrast row sums).
  - Host (numpy) does only small-vector glue. Sinkhorn never materializes
    the [K,N] matrix: with all entries positive, the iterate stays of the
    form Q_kn = u_k * E_nk * v_n, so each half-iteration is one matvec
    against E. Downstream only row-l1-normalized gathers of Q are needed,
    so the per-row scale (v) cancels and is never applied.
"""

import base64
import hashlib
import os
import sys
import time
import zlib

sys.path.insert(0, "/opt/trn_rl_repo")

import numpy as np

F32 = None  # filled lazily

NUM_CLASSES = 100
CACHE_SIZE = 50
K = NUM_CLASSES * CACHE_SIZE  # 5000
D = 128
B = 2048
V = 2
N = V * B  # 4096
NCORES = 8
NSH = N // NCORES  # 512
TEMP = 0.1
EPS = 0.05
PROTO_M = 0.99
TOPK = 10
SINK_ITERS = 3
CLS_PER_CORE = 13  # padded; cores 0-3 own 13 classes, 4-7 own 12 (+1 dup pad)
PROWS = CLS_PER_CORE * CACHE_SIZE  # 650

KCH = [(i * 512, 512) for i in range(9)] + [(4608, 392)]  # 5000 = 9*512+392

_TIMING = bool(os.environ.get("BASSK_TIMING"))
_DUMP_DIR = os.environ.get("BASSK_DUMP")

# Optionally filled with (sha256-of-bir-json, zlib+b64 NEFF bytes) so a fresh
# process can skip the BIR->NEFF compile entirely.
_EMBED_HASH = None
_EMBED_NEFF = None


def _tlog(msg, t0):
    if _TIMING:
        print(f"[bassk] {msg}: {time.perf_counter() - t0:.3f}s", file=sys.stderr, flush=True)
    return time.perf_counter()


def _build_nc():
    import contextlib

    import concourse.bass as bass
    from concourse import mybir

    F32 = mybir.dt.float32
    AFT = mybir.ActivationFunctionType
    ALU = mybir.AluOpType
    AX = mybir.AxisListType

    nc = bass.Bass(disable_frame_to_traceback=True)
    BF16 = mybir.dt.bfloat16
    fT = nc.declare_dram_parameter("featsT", [D, NSH], F32, isOutput=False)
    pT = nc.declare_dram_parameter("protosT", [D, K], F32, isOutput=False)
    km = nc.declare_dram_parameter("kmod", [1, K], F32, isOutput=False)
    lb = nc.declare_dram_parameter("labels4", [128, NSH // 128], F32, isOutput=False)
    mycl = nc.declare_dram_parameter("myclasses", [D, PROWS], F32, isOutput=False)
    csum = nc.declare_dram_parameter("classsums", [D, CLS_PER_CORE], F32, isOutput=False)
    itau_c = nc.declare_dram_parameter("invtaucol", [128, 1], F32, isOutput=False)
    beta_c = nc.declare_dram_parameter("betacol", [128, 1], F32, isOutput=False)
    E = nc.declare_dram_parameter("E", [NSH, K], BF16, isOutput=True)
    SE = nc.declare_dram_parameter("sumexp", [128, NSH // 128], F32, isOutput=True)
    PC = nc.declare_dram_parameter("pcon", [50, 2 * CLS_PER_CORE], F32, isOutput=True)

    km_ap = km[:]
    km_b = bass.AP(tensor=km_ap.tensor, offset=km_ap.offset, ap=[[0, 128]] + km_ap.ap[1:])

    es = contextlib.ExitStack()
    with es:
        fts = es.enter_context(nc.sbuf_tensor([D, NSH], F32))
        pts = es.enter_context(nc.sbuf_tensor([D, K], F32))
        kmt = es.enter_context(nc.sbuf_tensor([128, K], F32))
        lbt = es.enter_context(nc.sbuf_tensor([128, NSH // 128], F32))
        myt = es.enter_context(nc.sbuf_tensor([D, PROWS], F32))
        cst = es.enter_context(nc.sbuf_tensor([D, CLS_PER_CORE], F32))
        itt = es.enter_context(nc.sbuf_tensor([128, 1], F32))
        bct = es.enter_context(nc.sbuf_tensor([128, 1], F32))
        epsc = es.enter_context(nc.sbuf_tensor([128, 1], F32))
        onec = es.enter_context(nc.sbuf_tensor([128, 1], F32))
        e2 = es.enter_context(nc.sbuf_tensor([128, 512], BF16))
        lg = es.enter_context(nc.sbuf_tensor([128, 512], F32))
        mk = es.enter_context(nc.sbuf_tensor([128, 512], F32))
        pl = es.enter_context(nc.sbuf_tensor([128, 512], F32))
        ng = es.enter_context(nc.sbuf_tensor([128, 512], F32))
        t1 = es.enter_context(nc.sbuf_tensor([128, 512], F32))
        im = es.enter_context(nc.sbuf_tensor([128, 512], F32))
        tt = es.enter_context(nc.sbuf_tensor([128, 512], F32))
        ex = es.enter_context(nc.sbuf_tensor([128, 512], F32))
        nacc = es.enter_context(nc.sbuf_tensor([128, 1], F32))
        rr = es.enter_context(nc.sbuf_tensor([128, 1], F32))
        racc = es.enter_context(nc.sbuf_tensor([128, 1], F32))
        rc = es.enter_context(nc.sbuf_tensor([128, 1], F32))
        nacc4 = es.enter_context(nc.sbuf_tensor([128, NSH // 128], F32))
        pc_sb = es.enter_context(nc.sbuf_tensor([128, 2 * CLS_PER_CORE], F32))
        pA = es.enter_context(nc.psum_tensor([128, 512], F32))
        pB = es.enter_context(nc.psum_tensor([128, 1], F32))
        tok = es.enter_context(nc.semaphore())
        block = es.enter_context(nc.Block())

        # Ledger of (engine, emit_fn, inc, wait_override). Serial token chain:
        # entry i waits tok >= cum[i] (or wait_override) and incs by `inc`
        # (16 for DMA, 1 for compute). Output DMAs override their wait to the
        # producer's position so they stream in parallel with later compute.
        ledger = []

        def op(eng, fn, inc=1, wait_at=None):
            ledger.append([eng, fn, inc, wait_at])
            return len(ledger)  # 1-based index into ledger

        # ---- loads ----
        op("sync", lambda: nc.sync.dma_start(out=fts[:], in_=fT[:]), 16)
        op("sync", lambda: nc.sync.dma_start(out=pts[:], in_=pT[:]), 16)
        op("sync", lambda: nc.sync.dma_start(out=kmt[:], in_=km_b), 16)
        op("sync", lambda: nc.sync.dma_start(out=lbt[:], in_=lb[:]), 16)
        op("sync", lambda: nc.sync.dma_start(out=myt[:], in_=mycl[:]), 16)
        op("sync", lambda: nc.sync.dma_start(out=cst[:], in_=csum[:]), 16)
        op("sync", lambda: nc.sync.dma_start(out=itt[:], in_=itau_c[:]), 16)
        op("sync", lambda: nc.sync.dma_start(out=bct[:], in_=beta_c[:]), 16)
        op("dve", lambda: nc.vector.memset(epsc[:], 1e-10))
        op("dve", lambda: nc.vector.memset(onec[:], 1.0))

        # ---- per n-chunk: E, sumexp of total_logits ----
        for nb in range(NSH // 128):
            op("dve", lambda nb=nb: nc.vector.memset(nacc[:], 0.0))
            for k0, kw in KCH:
                op("pe", lambda nb=nb, k0=k0, kw=kw: nc.tensor.matmul(
                    pA[:, :kw], fts[:, nb * 128 : (nb + 1) * 128],
                    pts[:, k0 : k0 + kw], start=True, stop=True))
                prod = op("act", lambda kw=kw: nc.scalar.activation(
                    out=e2[:, :kw], in_=pA[:, :kw], func=AFT.Exp, scale=1.0 / EPS))
                op("sync", lambda nb=nb, k0=k0, kw=kw: nc.sync.dma_start(
                    out=E[nb * 128 : (nb + 1) * 128, k0 : k0 + kw], in_=e2[:, :kw]),
                    16, wait_at=prod)
                op("act", lambda kw=kw: nc.scalar.activation(
                    out=lg[:, :kw], in_=pA[:, :kw], func=AFT.Exp, scale=1.0 / TEMP))
                op("act", lambda kw=kw: nc.scalar.activation(
                    out=lg[:, :kw], in_=lg[:, :kw], func=AFT.Ln, bias=onec[:]))
                op("dve", lambda nb=nb, k0=k0, kw=kw: nc.vector.tensor_scalar(
                    out=mk[:, :kw], in0=kmt[:, k0 : k0 + kw],
                    scalar1=lbt[:, nb : nb + 1], scalar2=None, op0=ALU.is_equal))
                op("dve", lambda kw=kw: nc.vector.tensor_mul(
                    out=pl[:, :kw], in0=lg[:, :kw], in1=mk[:, :kw]))
                op("dve", lambda kw=kw: nc.vector.tensor_sub(
                    out=ng[:, :kw], in0=lg[:, :kw], in1=pl[:, :kw]))
                op("act", lambda kw=kw: nc.scalar.activation(
                    out=t1[:, :kw], in_=ng[:, :kw], func=AFT.Ln, bias=epsc[:]))
                op("act", lambda kw=kw: nc.scalar.activation(
                    out=im[:, :kw], in_=t1[:, :kw], func=AFT.Exp, scale=bct[:]))
                op("dve", lambda kw=kw: nc.vector.tensor_mul(
                    out=tt[:, :kw], in0=im[:, :kw], in1=ng[:, :kw]))
                op("dve", lambda kw=kw: nc.vector.tensor_add(
                    out=tt[:, :kw], in0=tt[:, :kw], in1=pl[:, :kw]))
                op("act", lambda kw=kw: nc.scalar.activation(
                    out=ex[:, :kw], in_=tt[:, :kw], func=AFT.Exp))
                op("dve", lambda kw=kw: nc.vector.reduce_sum(
                    out=rr[:], in_=ex[:, :kw], axis=AX.X))
                op("dve", lambda: nc.vector.tensor_add(
                    out=nacc[:], in0=nacc[:], in1=rr[:]))
            op("dve", lambda nb=nb: nc.vector.tensor_copy(
                out=nacc4[:, nb : nb + 1], in_=nacc[:]))

        # ---- proto-contrast shard ----
        for c in range(CLS_PER_CORE):
            op("dve", lambda: nc.vector.memset(racc[:50, :], 0.0))
            for k0, kw in KCH:
                op("pe", lambda c=c, k0=k0, kw=kw: nc.tensor.matmul(
                    pA[:50, :kw], myt[:, c * 50 : (c + 1) * 50],
                    pts[:, k0 : k0 + kw], start=True, stop=True))
                op("act", lambda kw=kw: nc.scalar.activation(
                    out=ex[:50, :kw], in_=pA[:50, :kw], func=AFT.Exp,
                    scale=itt[:50, :]))
                op("dve", lambda kw=kw: nc.vector.reduce_sum(
                    out=rc[:50, :], in_=ex[:50, :kw], axis=AX.X))
                op("dve", lambda: nc.vector.tensor_add(
                    out=racc[:50, :], in0=racc[:50, :], in1=rc[:50, :]))
            op("pe", lambda c=c: nc.tensor.matmul(
                pB[:50, :], myt[:, c * 50 : (c + 1) * 50], cst[:, c : c + 1],
                start=True, stop=True))
            op("dve", lambda c=c: nc.vector.tensor_copy(
                out=pc_sb[:50, 2 * c : 2 * c + 1], in_=racc[:50, :]))
            op("dve", lambda c=c: nc.vector.tensor_copy(
                out=pc_sb[:50, 2 * c + 1 : 2 * c + 2], in_=pB[:50, :]))

        op("sync", lambda: nc.sync.dma_start(out=SE[:], in_=nacc4[:]), 16)
        op("sync", lambda: nc.sync.dma_start(out=PC[:], in_=pc_sb[:50, :]), 16)

        # cumulative token thresholds
        cum = [0]
        for _, _, inc, _ in ledger:
            cum.append(cum[-1] + inc)
        total = cum[-1]

        streams = {"sync": [], "pe": [], "act": [], "dve": []}
        prev_eng = None
        for i, (eng, fn, inc, wait_at) in enumerate(ledger):
            thresh = cum[wait_at] if wait_at is not None else cum[i]
            need_wait = (eng != prev_eng) or (wait_at is not None)
            streams[eng].append((need_wait, thresh, fn, inc))
            if wait_at is None:
                prev_eng = eng

        def emit(eng_obj, name):
            last_wait = -1
            for need_wait, thresh, fn, inc in streams[name]:
                if need_wait and thresh > last_wait:
                    eng_obj.wait_ge(tok, thresh)
                    last_wait = thresh
                fn().then_inc(tok, inc)

        @block.sync
        def _(sync):
            emit(sync, "sync")
            sync.wait_ge(tok, total)

        @block.tensor
        def _(pe):
            emit(pe, "pe")

        @block.scalar
        def _(act):
            emit(act, "act")

        @block.vector
        def _(dve):
            emit(dve, "dve")

    return nc


def _patch_compiler(bass2jax):
    """Wrap bass2jax.compile_bir_kernel: serve the embedded NEFF on a hash
    match (skips the walrus compile in a fresh process); optionally dump the
    (bir_json, neff) pair so it can be embedded."""
    if getattr(bass2jax, "_bassk_patched", False):
        return
    orig = bass2jax.compile_bir_kernel

    def patched(bir_json, tmpdir, neff_name="file.neff"):
        h = hashlib.sha256(bir_json).hexdigest()
        if _EMBED_HASH is not None and h == _EMBED_HASH:
            path = os.path.join(tmpdir, neff_name)
            with open(path, "wb") as f:
                f.write(zlib.decompress(base64.b64decode(_EMBED_NEFF)))
            return path
        t0 = time.perf_counter()
        path = orig(bir_json, tmpdir, neff_name)
        _tlog(f"walrus compile (hash {h[:12]})", t0)
        if _DUMP_DIR:
            import shutil

            os.makedirs(_DUMP_DIR, exist_ok=True)
            with open(os.path.join(_DUMP_DIR, f"{h}.bir.json"), "wb") as f:
                f.write(bir_json)
            shutil.copy(path, os.path.join(_DUMP_DIR, f"{h}.neff"))
        return path

    bass2jax.compile_bir_kernel = patched
    bass2jax._bassk_patched = True


_RT = {}


def _runtime():
    if _RT:
        return _RT
    t0 = time.perf_counter()
    import jax
    from jax.experimental.shard_map import shard_map
    from jax.sharding import Mesh, PartitionSpec

    from concourse import bass2jax, mybir

    t0 = _tlog("imports", t0)
    nc = _build_nc()
    t0 = _tlog("build_nc", t0)

    bass2jax.install_neuronx_cc_hook()
    _patch_compiler(bass2jax)
    assert nc.dbg_addr is None
    partition_name = nc.partition_id_tensor.name if nc.partition_id_tensor else None

    in_names, out_names, out_avals = [], [], []
    for alloc in nc.m.functions[0].allocations:
        if not isinstance(alloc, mybir.MemoryLocationSet):
            continue
        name = alloc.memorylocations[0].name
        if alloc.kind == "ExternalInput":
            if name != partition_name:
                in_names.append(name)
        elif alloc.kind == "ExternalOutput":
            shape = tuple(alloc.tensor_shape)
            dtype = mybir.dt.np(alloc.dtype)
            out_avals.append(jax.core.ShapedArray(shape, dtype))
            out_names.append(name)
    n_params = len(in_names)
    n_outs = len(out_names)
    # Outputs are fully written by the NEFF, so no pre-zeroed donated output
    # operands are needed (upstream ships 82MB of zeros per call for kernels
    # that partially write their outputs).
    all_names = list(in_names)
    if partition_name is not None:
        all_names = all_names + [partition_name]

    def _body(*args):
        operands = list(args)
        if partition_name is not None:
            operands.append(bass2jax.partition_id_tensor())
        outs = bass2jax._bass_exec_p.bind(
            *operands,
            out_avals=tuple(out_avals),
            in_names=tuple(all_names),
            out_names=tuple(out_names),
            lowering_input_output_aliases=(),
            sim_require_finite=True,
            sim_require_nnan=True,
            nc=nc,
        )
        return tuple(outs)

    devices = jax.devices()[:NCORES]
    assert len(devices) == NCORES
    mesh = Mesh(np.asarray(devices), ("core",))
    in_specs = (PartitionSpec("core"),) * n_params
    out_specs = (PartitionSpec("core"),) * n_outs
    sharded = jax.jit(
        shard_map(_body, mesh=mesh, in_specs=in_specs, out_specs=out_specs, check_rep=False),
        keep_unused=True,
    )
    _RT.update(
        fn=sharded,
        in_names=in_names,
        out_names=out_names,
        out_shapes=[tuple(a.shape) for a in out_avals],
        out_dtypes=[a.dtype for a in out_avals],
    )
    _tlog("runtime setup", t0)
    return _RT


def _run(in_maps):
    rt = _runtime()
    t0 = time.perf_counter()
    concat_in = [
        np.concatenate([np.asarray(m[name]) for m in in_maps], axis=0)
        for name in rt["in_names"]
    ]
    t0 = _tlog("concat inputs", t0)
    if "compiled" not in rt:
        lowered = rt["fn"].lower(*concat_in)
        t0 = _tlog("jit lower", t0)
        rt["compiled"] = lowered.compile()
        t0 = _tlog("jit compile", t0)
    out_arrs = rt["compiled"](*concat_in)
    for a in out_arrs:
        a.block_until_ready()
    t0 = _tlog("device call", t0)
    # np.asarray on a sharded global array assembles serially (~40MB/s); pulling
    # the 8 per-device shards concurrently runs at tunnel line rate (~1.5GB/s).
    from concurrent.futures import ThreadPoolExecutor

    def _fetch(a):
        shards = sorted(a.addressable_shards, key=lambda s: s.index[0].start or 0)
        with ThreadPoolExecutor(NCORES) as tp:
            parts = list(tp.map(lambda s: np.asarray(s.data), shards))
        return np.stack(parts, axis=0)  # [NCORES, shard_rows, ...]

    out_np = [_fetch(a) for a in out_arrs]
    t0 = _tlog("fetch outputs", t0)
    return {name: out_np[i] for i, name in enumerate(rt["out_names"])}


def _l2norm(x, axis=-1):
    return x / np.clip(np.linalg.norm(x, axis=axis, keepdims=True), 1e-12, None)


def _sinkhorn_uv(E):
    """E=[N,K] positive. Reference sinkhorn's (Q*B_).T equals
    N * u[None,:] * E * v[:,None] after SINK_ITERS (row-norm, col-norm)
    pairs. The per-row scale v cancels in all downstream row-l1-normalized
    uses, so only u is returned and the final col matvec is skipped."""
    v = np.ones(E.shape[0], np.float32)
    u = None
    for it in range(SINK_ITERS):
        u = 1.0 / np.maximum(float(K) * (E.T @ v), 1e-30)
        if it < SINK_ITERS - 1:
            v = 1.0 / np.maximum(float(N) * (E @ u), 1e-30)
    return u


def kernel(features, targets, beta, protos):
    t_all = time.perf_counter()
    features = np.asarray(features, np.float32)
    targets_np = np.asarray(targets)
    beta = float(np.asarray(beta))
    protos = np.asarray(protos, np.float32)

    feats = np.ascontiguousarray(features.transpose(1, 0, 2).reshape(N, D))
    labels = np.tile(targets_np.astype(np.int64), V)

    # per-core class shards for proto-contrast (padded to 13 classes/core)
    cls_of_core = []
    start = 0
    for c in range(NCORES):
        ncls = 13 if c < 4 else 12
        cl = list(range(start, start + ncls))
        start += ncls
        while len(cl) < CLS_PER_CORE:
            cl.append(cl[-1])  # pad with duplicate; ignored on readout
        cls_of_core.append(cl)

    kmod = (np.arange(K) % NUM_CLASSES).astype(np.float32)[None, :]
    beta_col = np.full((128, 1), beta, np.float32)

    def make_maps(protosT_np, pn_by_class, csums, itau_val):
        maps = []
        itau_col = np.full((128, 1), itau_val, np.float32)
        for c in range(NCORES):
            sh = feats[c * NSH : (c + 1) * NSH]
            lb4 = labels[c * NSH : (c + 1) * NSH].astype(np.float32).reshape(NSH // 128, 128).T
            mycl = np.concatenate([pn_by_class[ci] for ci in cls_of_core[c]], axis=1)
            cs = np.stack([csums[:, ci] for ci in cls_of_core[c]], axis=1)
            maps.append(
                {
                    "featsT": np.ascontiguousarray(sh.T),
                    "protosT": protosT_np,
                    "kmod": kmod,
                    "labels4": np.ascontiguousarray(lb4),
                    "myclasses": np.ascontiguousarray(mycl.astype(np.float32)),
                    "classsums": np.ascontiguousarray(cs.astype(np.float32)),
                    "invtaucol": itau_col,
                    "betacol": beta_col,
                }
            )
        return maps

    # ---------- pass 1: E1 from protos ----------
    t0 = time.perf_counter()
    protosT = np.ascontiguousarray(protos.T)
    pn_by_class_dummy = [protosT[:, ci * 50 : (ci + 1) * 50] for ci in range(NUM_CLASSES)]
    csums_dummy = np.zeros((D, NUM_CLASSES), np.float32)
    maps1 = make_maps(protosT, pn_by_class_dummy, csums_dummy, 1.0)
    t0 = _tlog("prep pass1", t0)
    res1 = _run(maps1)
    E1 = res1["E"].reshape(N, K).astype(np.float32)
    t0 = _tlog("pass1 total", t0)

    # ---------- host: sinkhorn 1 + top-k + proto update ----------
    u1 = _sinkhorn_uv(E1)
    t0 = _tlog("sinkhorn1", t0)
    cand_cols = (labels[:, None] + NUM_CLASSES * np.arange(CACHE_SIZE)[None, :]).astype(np.int64)
    rows = np.arange(N)[:, None]
    # row scale of Q1 cancels under top-k + row l1-norm; use u1*E1 gathered.
    E1g = np.take_along_axis(E1, cand_cols, axis=1)  # [N, 50]
    cand = E1g.astype(np.float64) * u1[cand_cols]
    top_j = np.argpartition(-cand, TOPK - 1, axis=1)[:, :TOPK]  # [N, 10]
    sel_cols = np.take_along_axis(cand_cols, top_j, axis=1)  # [N, 10]
    sel_vals = np.take_along_axis(cand, top_j, axis=1)  # [N, 10]
    um_w = sel_vals / np.clip(np.abs(sel_vals).sum(axis=1, keepdims=True), 1e-12, None)

    # scatter: uf[k] = sum over (n,j) with sel_cols==k of um_w*feats[n]
    cols = sel_cols.ravel()
    df = (um_w[..., None].astype(np.float32) * feats[:, None, :]).reshape(-1, D)
    order = np.argsort(cols, kind="stable")
    sc = cols[order]
    dfs = df[order]
    bounds = np.flatnonzero(np.r_[True, sc[1:] != sc[:-1]])
    sums = np.add.reduceat(dfs.astype(np.float64), bounds, axis=0)
    uf = np.zeros((K, D), np.float64)
    uf[sc[bounds]] = sums

    protos_new = PROTO_M * protos.astype(np.float64) + (1.0 - PROTO_M) * uf
    sim_mean = float(
        np.mean(np.sum(_l2norm(uf, 1) * _l2norm(protos.astype(np.float64), 1), axis=1))
    )
    protos2 = _l2norm(protos_new, axis=1)  # [K, D]
    tau = (1.0 + (0.5 - sim_mean)) * 0.4
    itau_val = 1.0 / tau
    t0 = _tlog("host mid", t0)

    # ---------- pass 2: E2 / sumexp / pcon from protos2 ----------
    protos2T = np.ascontiguousarray(protos2.T.astype(np.float32))
    pn_by_class = [
        np.ascontiguousarray(protos2T[:, ci::NUM_CLASSES]) for ci in range(NUM_CLASSES)
    ]  # each [D, 50]: columns of class ci
    csums = np.stack([pb.sum(axis=1) for pb in pn_by_class], axis=1)  # [D, 100]
    maps2 = make_maps(protos2T, pn_by_class, csums, itau_val)
    t0 = _tlog("prep pass2", t0)
    res2 = _run(maps2)
    E2 = res2["E"].reshape(N, K).astype(np.float32)
    sumexp = res2["sumexp"].transpose(0, 2, 1).reshape(N)
    t0 = _tlog("pass2 total", t0)

    # ---------- host: sinkhorn 2, mle, pcon ----------
    u2 = _sinkhorn_uv(E2)
    t0 = _tlog("sinkhorn2", t0)
    E2g = np.take_along_axis(E2, sel_cols, axis=1).astype(np.float64)  # [N, 10]
    lm_vals = E2g * u2[sel_cols]  # row scale cancels in l1norm
    lm_w = lm_vals / np.clip(np.abs(lm_vals).sum(axis=1, keepdims=True), 1e-12, None)
    out2_sel = EPS * np.log(np.clip(E2g, 1e-300, None))
    logit_sel = np.logaddexp(0.0, out2_sel / TEMP)
    pos = np.sum(lm_w * logit_sel, axis=1)  # [N]
    neg = np.log(np.clip(sumexp.astype(np.float64), 1e-300, None))  # [N]
    mle = -(pos.mean() - neg.mean())

    # pcon from per-proto row sums (permuted class-major order on device)
    rs = np.zeros(K, np.float64)  # sum_j exp(adc_ij)
    bs = np.zeros(K, np.float64)  # sum_{j in class block} raw_ij
    for c in range(NCORES):
        pc = res2["pcon"][c].astype(np.float64)  # [50, 2*CLS_PER_CORE]
        ncls = 13 if c < 4 else 12
        for slot, ci in enumerate(cls_of_core[c][:ncls]):
            rows_k = np.arange(K)[np.arange(K) % NUM_CLASSES == ci]  # original indices
            rs[rows_k] = pc[:, 2 * slot]
            bs[rows_k] = pc[:, 2 * slot + 1]
    inv_tau = 1.0 / tau
    ppos = (bs * inv_tau - inv_tau) / (CACHE_SIZE - 1) - inv_tau
    pneg = np.log(np.clip(rs - np.exp(inv_tau), 1e-300, None)) - inv_tau
    pcon = -(ppos - pneg).mean()
    _tlog("host tail", t0)
    _tlog("kernel total", t_all)

    return np.float32(mle + pcon)


# revision 20
# speedup vs baseline: 1.2912x; 1.2912x over previous
"""Trainium2 Bass kernel for nn_ADAPT_19748259627479 (PaCo-style loss_fn).

Strategy (8 NeuronCores, data-parallel over N=V*B=4096):
  - Each core owns a 512-row shard of feats (view-major) and computes the
    heavy [512,5000] work on-device: feats@protos.T matmuls, exp(./eps),
    softplus logits, the total_logits/imp elementwise chain + row sum-exp,
    and a 625-row shard of the proto-contrast exp(pn@pn.T/tau) reductions.
  - One NEFF (beta passed as an input column, so nothing input-dependent is
    baked in), compiled once per process and executed twice: pass 1 with
    protos (yields E1 for Sinkhorn 1), pass 2 with protos2 (yields E2, row
    sumexp for mle's neg term, and the proto-contrast row sums).
  - Host (numpy) does only small-vector glue. Sinkhorn never materializes
    the [K,N] matrix: with all entries positive, the iterate stays of the
    form Q_kn = u_k * E_nk * v_n, so each half-iteration is one matvec
    against E. Downstream only row-l1-normalized gathers of Q are needed,
    so the per-row scale (v) cancels and is never applied.
"""

import base64
import hashlib
import os
import sys
import time
import zlib

sys.path.insert(0, "/opt/trn_rl_repo")

import numpy as np

F32 = None  # filled lazily

NUM_CLASSES = 100
CACHE_SIZE = 50
K = NUM_CLASSES * CACHE_SIZE  # 5000
D = 128
B = 2048
V = 2
N = V * B  # 4096
NCORES = 8
NSH = N // NCORES  # 512
TEMP = 0.1
EPS = 0.05
PROTO_M = 0.99
TOPK = 10
SINK_ITERS = 3
CLS_PER_CORE = 13  # padded; cores 0-3 own 13 classes, 4-7 own 12 (+1 dup pad)
PROWS = CLS_PER_CORE * CACHE_SIZE  # 650

KCH = [(i * 512, 512) for i in range(9)] + [(4608, 392)]  # 5000 = 9*512+392

_TIMING = bool(os.environ.get("BASSK_TIMING"))
_DUMP_DIR = os.environ.get("BASSK_DUMP")

# Optionally filled with (sha256-of-bir-json, zlib+b64 NEFF bytes) so a fresh
# process can skip the BIR->NEFF compile entirely.
_EMBED_HASH = None
_EMBED_NEFF = None


def _tlog(msg, t0):
    if _TIMING:
        print(f"[bassk] {msg}: {time.perf_counter() - t0:.3f}s", file=sys.stderr, flush=True)
    return time.perf_counter()


def _build_nc():
    import contextlib

    import concourse.bass as bass
    from concourse import mybir

    F32 = mybir.dt.float32
    AFT = mybir.ActivationFunctionType
    ALU = mybir.AluOpType
    AX = mybir.AxisListType

    nc = bass.Bass(disable_frame_to_traceback=True)
    BF16 = mybir.dt.bfloat16
    fT = nc.declare_dram_parameter("featsT", [D, NSH], BF16, isOutput=False)
    pT = nc.declare_dram_parameter("protosT", [D, K], BF16, isOutput=False)
    km = nc.declare_dram_parameter("kmod", [1, K], F32, isOutput=False)
    lb = nc.declare_dram_parameter("labels4", [128, NSH // 128], F32, isOutput=False)
    mycl = nc.declare_dram_parameter("myclasses", [D, PROWS], BF16, isOutput=False)
    csum = nc.declare_dram_parameter("classsums", [D, CLS_PER_CORE], BF16, isOutput=False)
    itau_c = nc.declare_dram_parameter("invtaucol", [128, 1], F32, isOutput=False)
    beta_c = nc.declare_dram_parameter("betacol", [128, 1], F32, isOutput=False)
    E = nc.declare_dram_parameter("E", [NSH, K], BF16, isOutput=True)
    SE = nc.declare_dram_parameter("sumexp", [128, NSH // 128], F32, isOutput=True)
    PC = nc.declare_dram_parameter("pcon", [50, 2 * CLS_PER_CORE], F32, isOutput=True)

    km_ap = km[:]
    km_b = bass.AP(tensor=km_ap.tensor, offset=km_ap.offset, ap=[[0, 128]] + km_ap.ap[1:])

    es = contextlib.ExitStack()
    with es:
        fts = es.enter_context(nc.sbuf_tensor([D, NSH], BF16))
        pts = es.enter_context(nc.sbuf_tensor([D, K], BF16))
        kmt = es.enter_context(nc.sbuf_tensor([128, K], F32))
        lbt = es.enter_context(nc.sbuf_tensor([128, NSH // 128], F32))
        myt = es.enter_context(nc.sbuf_tensor([D, PROWS], BF16))
        cst = es.enter_context(nc.sbuf_tensor([D, CLS_PER_CORE], BF16))
        itt = es.enter_context(nc.sbuf_tensor([128, 1], F32))
        bct = es.enter_context(nc.sbuf_tensor([128, 1], F32))
        epsc = es.enter_context(nc.sbuf_tensor([128, 1], F32))
        onec = es.enter_context(nc.sbuf_tensor([128, 1], F32))
        e2 = es.enter_context(nc.sbuf_tensor([128, 512], BF16))
        lg = es.enter_context(nc.sbuf_tensor([128, 512], F32))
        mk = es.enter_context(nc.sbuf_tensor([128, 512], F32))
        pl = es.enter_context(nc.sbuf_tensor([128, 512], F32))
        ng = es.enter_context(nc.sbuf_tensor([128, 512], F32))
        t1 = es.enter_context(nc.sbuf_tensor([128, 512], F32))
        im = es.enter_context(nc.sbuf_tensor([128, 512], F32))
        tt = es.enter_context(nc.sbuf_tensor([128, 512], F32))
        ex = es.enter_context(nc.sbuf_tensor([128, 512], F32))
        nacc = es.enter_context(nc.sbuf_tensor([128, 1], F32))
        rr = es.enter_context(nc.sbuf_tensor([128, 1], F32))
        racc = es.enter_context(nc.sbuf_tensor([128, 1], F32))
        rc = es.enter_context(nc.sbuf_tensor([128, 1], F32))
        nacc4 = es.enter_context(nc.sbuf_tensor([128, NSH // 128], F32))
        pc_sb = es.enter_context(nc.sbuf_tensor([128, 2 * CLS_PER_CORE], F32))
        pA = es.enter_context(nc.psum_tensor([128, 512], F32))
        pB = es.enter_context(nc.psum_tensor([128, 1], F32))
        tok = es.enter_context(nc.semaphore())
        block = es.enter_context(nc.Block())

        # Ledger of (engine, emit_fn, inc, wait_override). Serial token chain:
        # entry i waits tok >= cum[i] (or wait_override) and incs by `inc`
        # (16 for DMA, 1 for compute). Output DMAs override their wait to the
        # producer's position so they stream in parallel with later compute.
        ledger = []

        def op(eng, fn, inc=1, wait_at=None):
            ledger.append([eng, fn, inc, wait_at])
            return len(ledger)  # 1-based index into ledger

        # ---- loads ----
        op("sync", lambda: nc.sync.dma_start(out=fts[:], in_=fT[:]), 16)
        op("sync", lambda: nc.sync.dma_start(out=pts[:], in_=pT[:]), 16)
        op("sync", lambda: nc.sync.dma_start(out=kmt[:], in_=km_b), 16)
        op("sync", lambda: nc.sync.dma_start(out=lbt[:], in_=lb[:]), 16)
        op("sync", lambda: nc.sync.dma_start(out=myt[:], in_=mycl[:]), 16)
        op("sync", lambda: nc.sync.dma_start(out=cst[:], in_=csum[:]), 16)
        op("sync", lambda: nc.sync.dma_start(out=itt[:], in_=itau_c[:]), 16)
        op("sync", lambda: nc.sync.dma_start(out=bct[:], in_=beta_c[:]), 16)
        op("dve", lambda: nc.vector.memset(epsc[:], 1e-10))
        op("dve", lambda: nc.vector.memset(onec[:], 1.0))

        # ---- per n-chunk: E, sumexp of total_logits ----
        for nb in range(NSH // 128):
            op("dve", lambda nb=nb: nc.vector.memset(nacc[:], 0.0))
            for k0, kw in KCH:
                op("pe", lambda nb=nb, k0=k0, kw=kw: nc.tensor.matmul(
                    pA[:, :kw], fts[:, nb * 128 : (nb + 1) * 128],
                    pts[:, k0 : k0 + kw], start=True, stop=True))
                prod = op("act", lambda kw=kw: nc.scalar.activation(
                    out=e2[:, :kw], in_=pA[:, :kw], func=AFT.Exp, scale=1.0 / EPS))
                op("sync", lambda nb=nb, k0=k0, kw=kw: nc.sync.dma_start(
                    out=E[nb * 128 : (nb + 1) * 128, k0 : k0 + kw], in_=e2[:, :kw]),
                    16, wait_at=prod)
                op("act", lambda kw=kw: nc.scalar.activation(
                    out=lg[:, :kw], in_=pA[:, :kw], func=AFT.Exp, scale=1.0 / TEMP))
                op("act", lambda kw=kw: nc.scalar.activation(
                    out=lg[:, :kw], in_=lg[:, :kw], func=AFT.Ln, bias=onec[:]))
                op("dve", lambda nb=nb, k0=k0, kw=kw: nc.vector.tensor_scalar(
                    out=mk[:, :kw], in0=kmt[:, k0 : k0 + kw],
                    scalar1=lbt[:, nb : nb + 1], scalar2=None, op0=ALU.is_equal))
                op("dve", lambda kw=kw: nc.vector.tensor_mul(
                    out=pl[:, :kw], in0=lg[:, :kw], in1=mk[:, :kw]))
                op("dve", lambda kw=kw: nc.vector.tensor_sub(
                    out=ng[:, :kw], in0=lg[:, :kw], in1=pl[:, :kw]))
                op("act", lambda kw=kw: nc.scalar.activation(
                    out=t1[:, :kw], in_=ng[:, :kw], func=AFT.Ln, bias=epsc[:]))
                op("act", lambda kw=kw: nc.scalar.activation(
                    out=im[:, :kw], in_=t1[:, :kw], func=AFT.Exp, scale=bct[:]))
                op("dve", lambda kw=kw: nc.vector.tensor_mul(
                    out=tt[:, :kw], in0=im[:, :kw], in1=ng[:, :kw]))
                op("dve", lambda kw=kw: nc.vector.tensor_add(
                    out=tt[:, :kw], in0=tt[:, :kw], in1=pl[:, :kw]))
                op("act", lambda kw=kw: nc.scalar.activation(
                    out=ex[:, :kw], in_=tt[:, :kw], func=AFT.Exp))
                op("dve", lambda kw=kw: nc.vector.reduce_sum(
                    out=rr[:], in_=ex[:, :kw], axis=AX.X))
                op("dve", lambda: nc.vector.tensor_add(
                    out=nacc[:], in0=nacc[:], in1=rr[:]))
            op("dve", lambda nb=nb: nc.vector.tensor_copy(
                out=nacc4[:, nb : nb + 1], in_=nacc[:]))

        # ---- proto-contrast shard ----
        for c in range(CLS_PER_CORE):
            op("dve", lambda: nc.vector.memset(racc[:50, :], 0.0))
            for k0, kw in KCH:
                op("pe", lambda c=c, k0=k0, kw=kw: nc.tensor.matmul(
                    pA[:50, :kw], myt[:, c * 50 : (c + 1) * 50],
                    pts[:, k0 : k0 + kw], start=True, stop=True))
                op("act", lambda kw=kw: nc.scalar.activation(
                    out=ex[:50, :kw], in_=pA[:50, :kw], func=AFT.Exp,
                    scale=itt[:50, :]))
                op("dve", lambda kw=kw: nc.vector.reduce_sum(
                    out=rc[:50, :], in_=ex[:50, :kw], axis=AX.X))
                op("dve", lambda: nc.vector.tensor_add(
                    out=racc[:50, :], in0=racc[:50, :], in1=rc[:50, :]))
            op("pe", lambda c=c: nc.tensor.matmul(
                pB[:50, :], myt[:, c * 50 : (c + 1) * 50], cst[:, c : c + 1],
                start=True, stop=True))
            op("dve", lambda c=c: nc.vector.tensor_copy(
                out=pc_sb[:50, 2 * c : 2 * c + 1], in_=racc[:50, :]))
            op("dve", lambda c=c: nc.vector.tensor_copy(
                out=pc_sb[:50, 2 * c + 1 : 2 * c + 2], in_=pB[:50, :]))

        op("sync", lambda: nc.sync.dma_start(out=SE[:], in_=nacc4[:]), 16)
        op("sync", lambda: nc.sync.dma_start(out=PC[:], in_=pc_sb[:50, :]), 16)

        # cumulative token thresholds
        cum = [0]
        for _, _, inc, _ in ledger:
            cum.append(cum[-1] + inc)
        total = cum[-1]

        streams = {"sync": [], "pe": [], "act": [], "dve": []}
        prev_eng = None
        for i, (eng, fn, inc, wait_at) in enumerate(ledger):
            thresh = cum[wait_at] if wait_at is not None else cum[i]
            need_wait = (eng != prev_eng) or (wait_at is not None)
            streams[eng].append((need_wait, thresh, fn, inc))
            if wait_at is None:
                prev_eng = eng

        def emit(eng_obj, name):
            last_wait = -1
            for need_wait, thresh, fn, inc in streams[name]:
                if need_wait and thresh > last_wait:
                    eng_obj.wait_ge(tok, thresh)
                    last_wait = thresh
                fn().then_inc(tok, inc)

        @block.sync
        def _(sync):
            emit(sync, "sync")
            sync.wait_ge(tok, total)

        @block.tensor
        def _(pe):
            emit(pe, "pe")

        @block.scalar
        def _(act):
            emit(act, "act")

        @block.vector
        def _(dve):
            emit(dve, "dve")

    return nc


def _patch_compiler(bass2jax):
    """Wrap bass2jax.compile_bir_kernel: serve the embedded NEFF on a hash
    match (skips the walrus compile in a fresh process); optionally dump the
    (bir_json, neff) pair so it can be embedded."""
    if getattr(bass2jax, "_bassk_patched", False):
        return
    orig = bass2jax.compile_bir_kernel

    def patched(bir_json, tmpdir, neff_name="file.neff"):
        h = hashlib.sha256(bir_json).hexdigest()
        if _EMBED_HASH is not None and h == _EMBED_HASH:
            path = os.path.join(tmpdir, neff_name)
            with open(path, "wb") as f:
                f.write(zlib.decompress(base64.b64decode(_EMBED_NEFF)))
            return path
        t0 = time.perf_counter()
        path = orig(bir_json, tmpdir, neff_name)
        _tlog(f"walrus compile (hash {h[:12]})", t0)
        if _DUMP_DIR:
            import shutil

            os.makedirs(_DUMP_DIR, exist_ok=True)
            with open(os.path.join(_DUMP_DIR, f"{h}.bir.json"), "wb") as f:
                f.write(bir_json)
            shutil.copy(path, os.path.join(_DUMP_DIR, f"{h}.neff"))
        return path

    bass2jax.compile_bir_kernel = patched
    bass2jax._bassk_patched = True


_RT = {}


def _runtime():
    if _RT:
        return _RT
    t0 = time.perf_counter()
    import jax
    from jax.experimental.shard_map import shard_map
    from jax.sharding import Mesh, PartitionSpec

    from concourse import bass2jax, mybir

    t0 = _tlog("imports", t0)
    nc = _build_nc()
    t0 = _tlog("build_nc", t0)

    bass2jax.install_neuronx_cc_hook()
    _patch_compiler(bass2jax)
    assert nc.dbg_addr is None
    partition_name = nc.partition_id_tensor.name if nc.partition_id_tensor else None

    in_names, out_names, out_avals = [], [], []
    for alloc in nc.m.functions[0].allocations:
        if not isinstance(alloc, mybir.MemoryLocationSet):
            continue
        name = alloc.memorylocations[0].name
        if alloc.kind == "ExternalInput":
            if name != partition_name:
                in_names.append(name)
        elif alloc.kind == "ExternalOutput":
            shape = tuple(alloc.tensor_shape)
            dtype = mybir.dt.np(alloc.dtype)
            out_avals.append(jax.core.ShapedArray(shape, dtype))
            out_names.append(name)
    n_params = len(in_names)
    n_outs = len(out_names)
    # Outputs are fully written by the NEFF, so no pre-zeroed donated output
    # operands are needed (upstream ships 82MB of zeros per call for kernels
    # that partially write their outputs).
    all_names = list(in_names)
    if partition_name is not None:
        all_names = all_names + [partition_name]

    def _body(*args):
        operands = list(args)
        if partition_name is not None:
            operands.append(bass2jax.partition_id_tensor())
        outs = bass2jax._bass_exec_p.bind(
            *operands,
            out_avals=tuple(out_avals),
            in_names=tuple(all_names),
            out_names=tuple(out_names),
            lowering_input_output_aliases=(),
            sim_require_finite=True,
            sim_require_nnan=True,
            nc=nc,
        )
        return tuple(outs)

    devices = jax.devices()[:NCORES]
    assert len(devices) == NCORES
    mesh = Mesh(np.asarray(devices), ("core",))
    in_specs = (PartitionSpec("core"),) * n_params
    out_specs = (PartitionSpec("core"),) * n_outs
    sharded = jax.jit(
        shard_map(_body, mesh=mesh, in_specs=in_specs, out_specs=out_specs, check_rep=False),
        keep_unused=True,
    )
    _RT.update(
        fn=sharded,
        in_names=in_names,
        out_names=out_names,
        out_shapes=[tuple(a.shape) for a in out_avals],
        out_dtypes=[a.dtype for a in out_avals],
    )
    _tlog("runtime setup", t0)
    return _RT


def _run(in_maps):
    rt = _runtime()
    t0 = time.perf_counter()
    concat_in = [
        np.concatenate([np.asarray(m[name]) for m in in_maps], axis=0)
        for name in rt["in_names"]
    ]
    t0 = _tlog("concat inputs", t0)
    if "compiled" not in rt:
        lowered = rt["fn"].lower(*concat_in)
        t0 = _tlog("jit lower", t0)
        rt["compiled"] = lowered.compile()
        t0 = _tlog("jit compile", t0)
    out_arrs = rt["compiled"](*concat_in)
    for a in out_arrs:
        a.block_until_ready()
    t0 = _tlog("device call", t0)
    # np.asarray on a sharded global array assembles serially (~40MB/s); pulling
    # the 8 per-device shards concurrently runs at tunnel line rate (~1.5GB/s).
    from concurrent.futures import ThreadPoolExecutor

    def _fetch(a):
        shards = sorted(a.addressable_shards, key=lambda s: s.index[0].start or 0)
        with ThreadPoolExecutor(NCORES) as tp:
            parts = list(tp.map(lambda s: np.asarray(s.data), shards))
        return np.stack(parts, axis=0)  # [NCORES, shard_rows, ...]

    out_np = [_fetch(a) for a in out_arrs]
    t0 = _tlog("fetch outputs", t0)
    return {name: out_np[i] for i, name in enumerate(rt["out_names"])}


def _l2norm(x, axis=-1):
    return x / np.clip(np.linalg.norm(x, axis=axis, keepdims=True), 1e-12, None)


def _sinkhorn_uv(E):
    """E=[N,K] positive. Reference sinkhorn's (Q*B_).T equals
    N * u[None,:] * E * v[:,None] after SINK_ITERS (row-norm, col-norm)
    pairs. The per-row scale v cancels in all downstream row-l1-normalized
    uses, so only u is returned and the final col matvec is skipped."""
    v = np.ones(E.shape[0], np.float32)
    u = None
    for it in range(SINK_ITERS):
        u = 1.0 / np.maximum(float(K) * (E.T @ v), 1e-30)
        if it < SINK_ITERS - 1:
            v = 1.0 / np.maximum(float(N) * (E @ u), 1e-30)
    return u


def kernel(features, targets, beta, protos):
    t_all = time.perf_counter()
    features = np.asarray(features, np.float32)
    targets_np = np.asarray(targets)
    beta = float(np.asarray(beta))
    protos = np.asarray(protos, np.float32)

    feats = np.ascontiguousarray(features.transpose(1, 0, 2).reshape(N, D))
    labels = np.tile(targets_np.astype(np.int64), V)

    # per-core class shards for proto-contrast (padded to 13 classes/core)
    cls_of_core = []
    start = 0
    for c in range(NCORES):
        ncls = 13 if c < 4 else 12
        cl = list(range(start, start + ncls))
        start += ncls
        while len(cl) < CLS_PER_CORE:
            cl.append(cl[-1])  # pad with duplicate; ignored on readout
        cls_of_core.append(cl)

    import ml_dtypes

    bf16 = ml_dtypes.bfloat16
    kmod = (np.arange(K) % NUM_CLASSES).astype(np.float32)[None, :]
    beta_col = np.full((128, 1), beta, np.float32)

    def make_maps(protosT_np, pn_by_class, csums, itau_val):
        maps = []
        itau_col = np.full((128, 1), itau_val, np.float32)
        protosT_bf = np.ascontiguousarray(protosT_np.astype(bf16))
        for c in range(NCORES):
            sh = feats[c * NSH : (c + 1) * NSH]
            lb4 = labels[c * NSH : (c + 1) * NSH].astype(np.float32).reshape(NSH // 128, 128).T
            mycl = np.concatenate([pn_by_class[ci] for ci in cls_of_core[c]], axis=1)
            cs = np.stack([csums[:, ci] for ci in cls_of_core[c]], axis=1)
            maps.append(
                {
                    "featsT": np.ascontiguousarray(sh.T.astype(bf16)),
                    "protosT": protosT_bf,
                    "kmod": kmod,
                    "labels4": np.ascontiguousarray(lb4),
                    "myclasses": np.ascontiguousarray(mycl.astype(bf16)),
                    "classsums": np.ascontiguousarray(cs.astype(bf16)),
                    "invtaucol": itau_col,
                    "betacol": beta_col,
                }
            )
        return maps

    # ---------- pass 1: E1 from protos ----------
    t0 = time.perf_counter()
    protosT = np.ascontiguousarray(protos.T)
    pn_by_class_dummy = [protosT[:, ci * 50 : (ci + 1) * 50] for ci in range(NUM_CLASSES)]
    csums_dummy = np.zeros((D, NUM_CLASSES), np.float32)
    maps1 = make_maps(protosT, pn_by_class_dummy, csums_dummy, 1.0)
    t0 = _tlog("prep pass1", t0)
    res1 = _run(maps1)
    E1 = res1["E"].reshape(N, K).astype(np.float32)
    t0 = _tlog("pass1 total", t0)

    # ---------- host: sinkhorn 1 + top-k + proto update ----------
    u1 = _sinkhorn_uv(E1)
    t0 = _tlog("sinkhorn1", t0)
    cand_cols = (labels[:, None] + NUM_CLASSES * np.arange(CACHE_SIZE)[None, :]).astype(np.int64)
    rows = np.arange(N)[:, None]
    # row scale of Q1 cancels under top-k + row l1-norm; use u1*E1 gathered.
    E1g = np.take_along_axis(E1, cand_cols, axis=1)  # [N, 50]
    cand = E1g.astype(np.float64) * u1[cand_cols]
    top_j = np.argpartition(-cand, TOPK - 1, axis=1)[:, :TOPK]  # [N, 10]
    sel_cols = np.take_along_axis(cand_cols, top_j, axis=1)  # [N, 10]
    sel_vals = np.take_along_axis(cand, top_j, axis=1)  # [N, 10]
    um_w = sel_vals / np.clip(np.abs(sel_vals).sum(axis=1, keepdims=True), 1e-12, None)

    # scatter: uf[k] = sum over (n,j) with sel_cols==k of um_w*feats[n]
    cols = sel_cols.ravel()
    df = (um_w[..., None].astype(np.float32) * feats[:, None, :]).reshape(-1, D)
    order = np.argsort(cols, kind="stable")
    sc = cols[order]
    dfs = df[order]
    bounds = np.flatnonzero(np.r_[True, sc[1:] != sc[:-1]])
    sums = np.add.reduceat(dfs.astype(np.float64), bounds, axis=0)
    uf = np.zeros((K, D), np.float64)
    uf[sc[bounds]] = sums

    protos_new = PROTO_M * protos.astype(np.float64) + (1.0 - PROTO_M) * uf
    sim_mean = float(
        np.mean(np.sum(_l2norm(uf, 1) * _l2norm(protos.astype(np.float64), 1), axis=1))
    )
    protos2 = _l2norm(protos_new, axis=1)  # [K, D]
    tau = (1.0 + (0.5 - sim_mean)) * 0.4
    itau_val = 1.0 / tau
    t0 = _tlog("host mid", t0)

    # ---------- pass 2: E2 / sumexp / pcon from protos2 ----------
    protos2T = np.ascontiguousarray(protos2.T.astype(np.float32))
    pn_by_class = [
        np.ascontiguousarray(protos2T[:, ci::NUM_CLASSES]) for ci in range(NUM_CLASSES)
    ]  # each [D, 50]: columns of class ci
    csums = np.stack([pb.sum(axis=1) for pb in pn_by_class], axis=1)  # [D, 100]
    maps2 = make_maps(protos2T, pn_by_class, csums, itau_val)
    t0 = _tlog("prep pass2", t0)
    res2 = _run(maps2)
    E2 = res2["E"].reshape(N, K).astype(np.float32)
    sumexp = res2["sumexp"].transpose(0, 2, 1).reshape(N)
    t0 = _tlog("pass2 total", t0)

    # ---------- host: sinkhorn 2, mle, pcon ----------
    u2 = _sinkhorn_uv(E2)
    t0 = _tlog("sinkhorn2", t0)
    E2g = np.take_along_axis(E2, sel_cols, axis=1).astype(np.float64)  # [N, 10]
    lm_vals = E2g * u2[sel_cols]  # row scale cancels in l1norm
    lm_w = lm_vals / np.clip(np.abs(lm_vals).sum(axis=1, keepdims=True), 1e-12, None)
    out2_sel = EPS * np.log(np.clip(E2g, 1e-300, None))
    logit_sel = np.logaddexp(0.0, out2_sel / TEMP)
    pos = np.sum(lm_w * logit_sel, axis=1)  # [N]
    neg = np.log(np.clip(sumexp.astype(np.float64), 1e-300, None))  # [N]
    mle = -(pos.mean() - neg.mean())

    # pcon from per-proto row sums (permuted class-major order on device)
    rs = np.zeros(K, np.float64)  # sum_j exp(adc_ij)
    bs = np.zeros(K, np.float64)  # sum_{j in class block} raw_ij
    for c in range(NCORES):
        pc = res2["pcon"][c].astype(np.float64)  # [50, 2*CLS_PER_CORE]
        ncls = 13 if c < 4 else 12
        for slot, ci in enumerate(cls_of_core[c][:ncls]):
            rows_k = np.arange(K)[np.arange(K) % NUM_CLASSES == ci]  # original indices
            rs[rows_k] = pc[:, 2 * slot]
            bs[rows_k] = pc[:, 2 * slot + 1]
    inv_tau = 1.0 / tau
    ppos = (bs * inv_tau - inv_tau) / (CACHE_SIZE - 1) - inv_tau
    pneg = np.log(np.clip(rs - np.exp(inv_tau), 1e-300, None)) - inv_tau
    pcon = -(ppos - pneg).mean()
    _tlog("host tail", t0)
    _tlog("kernel total", t_all)

    return np.float32(mle + pcon)
